# revision 23
# baseline (speedup 1.0000x reference)
"""Cross-attention kernel for Trainium2, SPMD over 8 NeuronCores.

Problem: B=2, LQ=1024, LK=10000, E=256, H=8 heads of D=32.
  q = queries @ Wq + bq ; k = bev @ Wk + bk ; v = bev @ Wv + bv
  out = softmax(q k^T) v  @ Wo + bo

Sharding: core c -> (batch b = c // 4, head-pair hp = c % 4).  Each core
computes attention for its 2 heads of its batch plus the partial output
projection through its 64 rows of Wo.  Host sums the 4 partials per batch
and adds bo (plus the bv @ Wo term, see below).

Structural choices:
  - All hot-loop matmuls are 16-bit: q/k/v in fp16 (energies need the
    mantissa; fp16 streams 1 col/cycle vs 2 for fp32r), softmax weights in
    bf16 (exp values up to e^30 overflow fp16's range).  16-bit matmuls
    also feed the PE activity monitor, so no fp32 HAM-warm matmuls needed.
  - bk is dropped entirely (softmax is invariant to a per-query constant);
    bv rides through attention (weights sum to 1) and is folded into the
    host-side bias as bv @ Wo.  Both exact.
  - The softmax denominator comes from an all-ones column appended to v,
    so it falls out of the same PE matmuls that compute attn @ v.
  - exp() alternates per (kt, qc, head) unit between the Scalar engine
    (exact table exp) and the Vector engine computing a Schraudolph exp:
    bf16 bits as round(x * 2^7/ln2 + (127*2^7 - 7.35)) int16.  The -7.35
    debias makes the approximation mean-preserving so exact and
    approximate tiles mix without tilting the softmax average.
  - Energy PSUM tiles are per-head single banks in a 5-deep ring, so an
    exp only gates its own bank and the engines never co-idle waiting for
    a 2-bank group to drain.  k/v projection PSUM and the v-transpose
    PSUM share one further bank (temporally disjoint, same pool tag).
  - Energy matmuls (K=32) run as concurrent row-packed pairs (heads at PE
    rows 0-31 / 32-63); attn@v pairs are column-packed (output partitions
    0-32 / 64-96).
  - k/v projections share one stationary [Wk | Wv]; a single fp16
    [128, 512] PSUM evacuation per chunk yields both the k tile and the
    v^T tile, keeping the Vector engine free for exp.
  - Reciprocal of the 2048 denominators is done in a [128, 16] layout
    (tiny [1, 128] PE transposes in, 0-stride-broadcast matmuls out)
    instead of [1, 512] rows, which would run 30x slower on the per-lane
    DVE.
"""
import sys

sys.path.insert(0, "/opt/trn_rl_repo")

import numpy as np
import ml_dtypes

B, LQ, LK, E, H = 2, 1024, 10000, 256, 8
D = 32            # head dim
HPC = 2           # heads per core
DC = D * HPC      # 64 projected dims per core
LKP = 10240       # LK padded to a multiple of 512
NKT = LKP // 128  # 80 k-tiles
NKT_RUN = 79      # tile 79 is all padding (LK=10000 < 79*128)
NCH = LKP // 512  # 20 dma chunks

# Schraudolph exp constants (bf16 bits via int16).
SCH_A = float(2.0**7 / np.log(2.0))
SCH_B = float(127.0 * 128.0 - 7.35)

_CACHE = {}


def _build():
    import concourse.bacc as bacc
    import concourse.tile as tile
    from concourse import mybir

    F32 = mybir.dt.float32
    F16 = mybir.dt.float16
    BF16 = mybir.dt.bfloat16
    I16 = mybir.dt.int16
    AF = mybir.ActivationFunctionType
    ALU = mybir.AluOpType

    nc = bacc.Bacc("TRN2", target_bir_lowering=False)

    XQT = nc.dram_tensor("xqt", [128, 2, LQ], F16, kind="ExternalInput")
    XKT = nc.dram_tensor("xkt", [128, 2, LKP], F16, kind="ExternalInput")
    # all small constants packed into one byte tensor -> one DMA
    # layout (bytes): [0:512) wkv f16, [512:768) wq f16, [768:896) identb
    # f16, [896:1152) identbb bf16, [1152:1664) ident f32, [1664:1668) bq
    # f32 (rows 0-63), [1668:2692) wo bf16 (rows 0-31)
    # ... [2692:3204) wrm f32 (dense random data for the HAM feed)
    WPK = nc.dram_tensor("wpack", [128, 3204], mybir.dt.uint8,
                         kind="ExternalInput")
    # partial output, transposed: rows = embed dim, cols = query position
    OUT = nc.dram_tensor("out_t", [E, LQ], F32, kind="ExternalOutput")

    with tile.TileContext(nc) as tc:
        with (
            tc.tile_pool(name="singles", bufs=1) as sg,
            tc.tile_pool(name="stt", bufs=8) as stp,
            tc.tile_pool(name="ktp", bufs=2) as ktp,
            tc.tile_pool(name="avps", bufs=1, space="PSUM") as avp,
        ):
            # ---- inputs, ordered by first use (DMAs serialize on Sync) --
            wpk = sg.tile([128, 3204], mybir.dt.uint8, tag="wpk")
            nc.sync.dma_start(out=wpk, in_=WPK[:, :])
            xkT = sg.tile([128, 2, LKP], F16, tag="xkT")
            nc.sync.dma_start(out=xkT[:, :, 0:512], in_=XKT[:, :, 0:512])
            xqT = sg.tile([128, 2, LQ], F16, tag="xqT")
            nc.sync.dma_start(out=xqT, in_=XQT[:, :, :])
            wkv_r = wpk[:, 0:512].bitcast(F16).rearrange(
                "p (e c) -> p e c", e=2, c=128)
            wq_r = wpk[:, 512:768].bitcast(F16).rearrange(
                "p (e c) -> p e c", e=2, c=DC)
            identb = wpk[:, 768:896].bitcast(F16)
            identbb = wpk[:, 896:1152].bitcast(BF16)
            ident = wpk[:, 1152:1664].bitcast(F32)
            bq_sb = wpk[0:64, 1664:1668].bitcast(F32)
            wo_r = wpk[0:32, 1668:2692].bitcast(BF16).rearrange(
                "p (e c) -> p e c", e=2, c=E)

            # warm the ACT exp table before the steady loop (~2.7us load)
            dumm = sg.tile([64, 1], BF16, tag="dumm")
            nc.scalar.activation(dumm, bq_sb, AF.Exp)

            # random-data fp32 view for the HAM warm matmuls: the activity
            # monitor tracks actual fp32-path array toggling; 16-bit
            # matmuls never register, so the gate must be fed explicitly.
            # A wpack view (no copy) so the flip burst starts right after
            # the first DMA lands.
            wrm = wpk[:, 2692:3204].bitcast(F32)

            qT = sg.tile([64, LQ], F16, tag="qT")
            v_aug = sg.tile([128, NKT * 66], BF16, tag="vaug")
            # ones columns of v_aug (softmax-denominator trick)
            nc.vector.memset(
                v_aug[:, :].rearrange("p (k o) -> p k o", o=33)[:, :, 32:33],
                1.0)
            zz = sg.tile([1, 640], BF16, tag="zz")
            nc.vector.memset(zz, 0.0)

            av = {}
            kts = {}
            pending = []
            n_grp = [0]

            def warm(st, n):
                # HAM clock-gate feed: one plain-fp32 matmul inside every
                # ~3.4us window holds K=8/8.  Writes into a PSUM slot the
                # next start=True matmul overwrites, so it costs nothing.
                for _ in range(n):
                    nc.tensor.matmul(st[0:32, 0:128], wrm[:, 0:32],
                                     wrm[:, :], start=True, stop=True,
                                     skip_group_check=True)

            with (
                tc.tile_pool(name="stg0", bufs=1, space="PSUM") as ps0,
            ):
                # three persistent [128, 1024] energy tiles: a manual ring
                # (6 banks; + 2 attn accumulators = all 8).  One exp
                # instruction covers a whole group, and the k/v projection
                # borrows recently-freed ring tiles instead of owning a
                # bank.
                ring = [ps0.tile([128, 1024], F32, tag=f"st{i}",
                                 name=f"ring{i}") for i in range(3)]
                # ~4us dense fp32 burst to flip the HAM gate to K=8/8;
                # overlaps the prologue DMA chain (only needs wpack)
                warm(ring[2], 9)

                def dma_chunk(c):
                    cs = slice(c * 512, (c + 1) * 512)
                    nc.sync.dma_start(out=xkT[:, :, cs], in_=XKT[:, :, cs])

                def kv_stage(c):
                    # k/v projection into a borrowed ring-tile half, one
                    # stationary [Wk | Wv], one fp16 PSUM evacuation for
                    # both k (rows 0-63) and v^T
                    cs = slice(c * 512, (c + 1) * 512)
                    kv = ring[(n_grp[0] + 2) % 3][:, 0:512]
                    for e in range(2):
                        nc.tensor.matmul(kv, wkv_r[:, e, :], xkT[:, e, cs],
                                         start=(e == 0), stop=(e == 1))
                    kvt = ktp.tile([128, 512], F16, tag="kvt",
                                   name=f"kvt{c}")
                    nc.scalar.copy(kvt, kv)
                    kts[c] = kvt

                def v_stage(c):
                    # v^T -> v via PE transposes into another borrowed
                    # ring-tile region, then strided bf16 copy
                    kvt = kts[c]
                    vps = ring[(n_grp[0] + 2) % 3][:, 512:640].bitcast(F16)
                    for m in range(4):
                        nc.tensor.transpose(
                            vps[:, m * 64:(m + 1) * 64],
                            kvt[64:128, m * 128:(m + 1) * 128],
                            identb[64:128, :])
                    nc.vector.tensor_copy(
                        v_aug[:, c * 264:(c + 1) * 264].rearrange(
                            "p (k t o) -> p k t o", t=2, o=33)[:, :, :, 0:32],
                        vps[:, :].rearrange("p (k t d) -> p k t d", t=2, d=32))

                def flush_av(depth):
                    if len(pending) < depth:
                        return
                    kt, qc, sT = pending.pop(0)
                    for h in range(HPC):
                        # the K=1 zero matmul below initialized the whole
                        # bank, so every accumulation is start=False
                        nc.tensor.matmul(
                            av[qc][64 * h:64 * h + 33, :],
                            v_aug[:, kt * 66 + 33 * h:kt * 66 + 33 * h + 33],
                            sT[:, 512 * h:512 * (h + 1)],
                            start=False, stop=(kt == NKT_RUN - 1),
                            skip_group_check=True)

                def emit_group(kt, qc, ktile):
                    g = n_grp[0]
                    n_grp[0] += 1
                    st = ring[g % 3]
                    if g % 4 == 0:
                        warm(st, 1)
                    for h in range(HPC):
                        nc.tensor.matmul(
                            st[:, h * 512:(h + 1) * 512],
                            ktile[32 * h:32 * h + 32,
                                  (kt % 4) * 128:(kt % 4 + 1) * 128],
                            qT[32 * h:32 * h + 32, qc * 512:(qc + 1) * 512],
                            start=True, stop=True)
                    sT = stp.tile([128, 1024], BF16, tag="sT",
                                  name=f"sT{g}")
                    if (kt + qc) % 2 == 0:
                        nc.scalar.activation(sT, st, AF.Exp)
                    else:
                        nc.vector.tensor_scalar(
                            out=sT.bitcast(I16), in0=st,
                            scalar1=SCH_A, scalar2=SCH_B,
                            op0=ALU.mult, op1=ALU.add)
                    flush_av(3)
                    pending.append((kt, qc, sT))

                # ---- prologue ----
                for c in range(1, NCH):
                    dma_chunk(c)

                # q projection (borrows ring halves)
                for qc in range(2):
                    qp = ring[qc][:, 0:512]
                    for e in range(2):
                        nc.tensor.matmul(qp[0:64, :], wq_r[:, e, :],
                                         xqT[:, e, qc * 512:(qc + 1) * 512],
                                         start=(e == 0), stop=(e == 1))
                    nc.vector.tensor_scalar_add(
                        qT[:, qc * 512:(qc + 1) * 512], qp[0:64, :],
                        bq_sb[:, 0:1])

                kv_stage(0)
                v_stage(0)
                av[0] = avp.tile([128, 512], F32, tag="av_0", name="av_q0")
                av[1] = avp.tile([128, 512], F32, tag="av_1", name="av_q1")
                for qc in range(2):
                    # zero-fill the whole accumulator bank (K=1 matmul of
                    # zeros) so has_written covers all 128 partitions
                    nc.tensor.matmul(
                        av[qc][0:128, :], zz[0:1, 0:128], zz[0:1, 128:640],
                        start=True, stop=False, skip_group_check=True)

                # ---- steady state: software-pipelined by one chunk ----
                for c in range(NCH):
                    ktile = kts.pop(c)
                    i = 0
                    for j in range(4):
                        if c * 4 + j >= NKT_RUN:
                            break
                        for qc in range(2):
                            emit_group(c * 4 + j, qc, ktile)
                            if i == 3 and c + 1 < NCH:
                                kv_stage(c + 1)
                            if i == 5 and c + 1 < NCH:
                                v_stage(c + 1)
                            i += 1
                while pending:
                    flush_av(1)

            # =========== normalize + output projection ----
            attnT = sg.tile([32, 2, LQ], BF16, tag="attnT")
            out_sb = [sg.tile([128, LQ], F32, tag=f"out{e}", name=f"out{e}")
                      for e in range(2)]
            rT = sg.tile([128, 16], BF16, tag="rT")
            avs_t = {}

            with tc.tile_pool(name="scp", bufs=2, space="PSUM") as scp:
                # evacuate the [33, 512] accumulator slabs to partition
                # base 0 (rows 0-31 = dims, row 32 = denominator)
                for qc in range(2):
                    avs = sg.tile([33, 1024], BF16, tag=f"avs{qc}",
                                  name=f"avs{qc}")
                    for h in range(HPC):
                        if (qc + h) % 2 == 0:
                            nc.scalar.copy(avs[:, h * 512:(h + 1) * 512],
                                           av[qc][64 * h:64 * h + 33, :])
                        else:
                            nc.vector.tensor_copy(
                                avs[:, h * 512:(h + 1) * 512],
                                av[qc][64 * h:64 * h + 33, :])
                    avs_t[qc] = avs
                    warm(av[qc], 2)

                # transpose just the denominator rows into [128, 16] so
                # the reciprocal runs wide on the DVE
                # bf16 PSUM writes need 4-byte alignment: use even columns
                avT = scp.tile([128, 32], BF16, tag="avT", name="avT")
                for qc in range(2):
                    if qc == 1:
                        warm(av[0], 1)
                    for h in range(HPC):
                        for j in range(4):
                            idx = (qc * 2 + h) * 4 + j
                            nc.tensor.transpose(
                                avT[:, 2 * idx:2 * idx + 1],
                                avs_t[qc][32:33, h * 512 + j * 128:
                                          h * 512 + (j + 1) * 128],
                                identbb[32:33, 32:33])
                with nc.allow_low_precision(
                        reason="bf16 denominators: 0.4% rel, within budget"):
                    nc.vector.reciprocal(
                        rT, avT.rearrange("p (m o) -> p m o", o=2)[:, :, 0])

                for qc in range(2):
                    for h in range(HPC):
                        rb = scp.tile([32, 512], F32, tag="rb",
                                      name=f"rb{qc}{h}")
                        warm(rb, 1)
                        for j in range(4):
                            idx = (qc * 2 + h) * 4 + j
                            nc.tensor.matmul(
                                rb[:, j * 128:(j + 1) * 128],
                                rT[:, idx:idx + 1].broadcast_to((128, 32)),
                                identbb, start=True, stop=True)
                        nc.vector.tensor_mul(
                            attnT[:, h, qc * 512:(qc + 1) * 512],
                            avs_t[qc][0:32, h * 512:(h + 1) * 512], rb)

                for ec in range(2):
                    for qc in range(2):
                        po = scp.tile([128, 512], F32, tag="po",
                                      name=f"po{ec}{qc}")
                        if qc == 0:
                            warm(po, 1)
                        for h in range(HPC):
                            nc.tensor.matmul(
                                po, wo_r[:, h, ec * 128:(ec + 1) * 128],
                                attnT[:, h, qc * 512:(qc + 1) * 512],
                                start=(h == 0), stop=(h == 1))
                        if qc == 0:
                            nc.scalar.copy(out_sb[ec][:, 0:512], po)
                        else:
                            nc.vector.tensor_copy(out_sb[ec][:, 512:1024], po)
                    nc.sync.dma_start(out=OUT[ec * 128:(ec + 1) * 128, :],
                                      in_=out_sb[ec])

    nc.compile()
    return nc


def _get_nc():
    if "nc" not in _CACHE:
        _CACHE["nc"] = _build()
    return _CACHE["nc"]


def _host_in_maps(bev_emb, queries, Wq, bq, Wk, bk, Wv, bv, Wo, bo):
    bev_emb = np.asarray(bev_emb, dtype=np.float32)
    queries = np.asarray(queries, dtype=np.float32)
    Wq = np.asarray(Wq, dtype=np.float32)
    bq = np.asarray(bq, dtype=np.float32)
    Wk = np.asarray(Wk, dtype=np.float32)
    bk = np.asarray(bk, dtype=np.float32)
    Wv = np.asarray(Wv, dtype=np.float32)
    bv = np.asarray(bv, dtype=np.float32)
    Wo = np.asarray(Wo, dtype=np.float32)
    bo = np.asarray(bo, dtype=np.float32)

    BF = ml_dtypes.bfloat16
    ident = np.eye(128, dtype=np.float32)
    identb = np.zeros((128, 64), dtype=np.float16)
    identb[64:128] = np.eye(64, dtype=np.float16)
    identbb = np.eye(128, dtype=BF)

    # host-side layout staging (no flops): transposes + padding + casts
    xqt = []
    xkt = []
    for b in range(B):
        t = np.ascontiguousarray(
            queries[b].T.reshape(2, 128, LQ).transpose(1, 0, 2))
        xqt.append(t.astype(np.float16))
        kp = np.zeros((128, 2, LKP), dtype=np.float16)
        kp[:, :, :LK] = bev_emb[b].T.reshape(2, 128, LK).transpose(
            1, 0, 2).astype(np.float16)
        xkt.append(kp)

    in_maps = []
    for c in range(8):
        b, hp = c // 4, c % 4
        hs = slice(hp * DC, (hp + 1) * DC)
        wkv = np.concatenate([Wk[:, hs], Wv[:, hs]], axis=1)  # [256, 128]
        wpk = np.zeros((128, 3204), np.uint8)
        wpk[:, 0:512] = np.ascontiguousarray(
            wkv.reshape(2, 128, 128).transpose(1, 0, 2)).astype(
                np.float16).view(np.uint8).reshape(128, 512)
        wpk[:, 512:768] = np.ascontiguousarray(
            Wq[:, hs].reshape(2, 128, DC).transpose(1, 0, 2)).astype(
                np.float16).view(np.uint8).reshape(128, 256)
        wpk[:, 768:896] = identb.view(np.uint8).reshape(128, 128)
        wpk[:, 896:1152] = identbb.view(np.uint8).reshape(128, 256)
        wpk[:, 1152:1664] = ident.view(np.uint8).reshape(128, 512)
        wpk[0:64, 1664:1668] = np.ascontiguousarray(
            bq[hs]).astype(np.float32).view(np.uint8).reshape(64, 4)
        wpk[0:32, 1668:2692] = np.ascontiguousarray(
            Wo[hs, :].reshape(2, 32, E).transpose(1, 0, 2)).astype(
                BF).view(np.uint8).reshape(32, 1024)
        wpk[:, 2692:3204] = np.random.default_rng(7).standard_normal(
            (128, 128)).astype(np.float32).view(np.uint8).reshape(128, 512)
        in_maps.append({
            "xqt": xqt[b],
            "xkt": xkt[b],
            "wpack": wpk,
        })

    return in_maps


def kernel(bev_emb, queries, Wq, bq, Wk, bk, Wv, bv, Wo, bo):
    from concourse.bass_utils import run_bass_kernel_spmd

    in_maps = _host_in_maps(bev_emb, queries, Wq, bq, Wk, bk, Wv, bv, Wo, bo)
    nc = _get_nc()
    _CACHE["last_in_maps"] = in_maps
    res = run_bass_kernel_spmd(nc, in_maps, list(range(8)))
    _CACHE["last_result"] = res

    out = np.zeros((B, LQ, E), dtype=np.float32)
    for c in range(8):
        out[c // 4] += res.results[c]["out_t"].T
    # bk drops out of softmax exactly; bv rides through attention into the
    # output projection: out += bv @ Wo.  Both folded into the host bias.
    out += bo + bv @ Wo
    return out


# revision 24
# speedup vs baseline: 1.1346x; 1.1346x over previous
"""Cross-attention kernel for Trainium2, SPMD over 8 NeuronCores.

Problem: B=2, LQ=1024, LK=10000, E=256, H=8 heads of D=32.
  q = queries @ Wq + bq ; k = bev @ Wk + bk ; v = bev @ Wv + bv
  out = softmax(q k^T) v  @ Wo + bo

Sharding: core c -> (batch b = c // 4, head-pair hp = c % 4).  Each core
computes attention for its 2 heads of its batch plus the partial output
projection through its 64 rows of Wo.  Host sums the 4 partials per batch
and adds bo (plus the bv @ Wo term, see below).

Structural choices:
  - All hot-loop matmuls are 16-bit: q/k/v in fp16 (energies need the
    mantissa; fp16 streams 1 col/cycle vs 2 for fp32r), softmax weights in
    bf16 (exp values up to e^30 overflow fp16's range).  16-bit matmuls
    also feed the PE activity monitor, so no fp32 HAM-warm matmuls needed.
  - bk is dropped entirely (softmax is invariant to a per-query constant);
    bv rides through attention (weights sum to 1) and is folded into the
    host-side bias as bv @ Wo.  Both exact.
  - The softmax denominator comes from an all-ones column appended to v,
    so it falls out of the same PE matmuls that compute attn @ v.
  - exp() alternates per (kt, qc, head) unit between the Scalar engine
    (exact table exp) and the Vector engine computing a Schraudolph exp:
    bf16 bits as round(x * 2^7/ln2 + (127*2^7 - 7.35)) int16.  The -7.35
    debias makes the approximation mean-preserving so exact and
    approximate tiles mix without tilting the softmax average.
  - Energy PSUM tiles are per-head single banks in a 5-deep ring, so an
    exp only gates its own bank and the engines never co-idle waiting for
    a 2-bank group to drain.  k/v projection PSUM and the v-transpose
    PSUM share one further bank (temporally disjoint, same pool tag).
  - Energy matmuls (K=32) run as concurrent row-packed pairs (heads at PE
    rows 0-31 / 32-63); attn@v pairs are column-packed (output partitions
    0-32 / 64-96).
  - k/v projections share one stationary [Wk | Wv]; a single fp16
    [128, 512] PSUM evacuation per chunk yields both the k tile and the
    v^T tile, keeping the Vector engine free for exp.
  - Reciprocal of the 2048 denominators is done in a [128, 16] layout
    (tiny [1, 128] PE transposes in, 0-stride-broadcast matmuls out)
    instead of [1, 512] rows, which would run 30x slower on the per-lane
    DVE.
"""
import sys

sys.path.insert(0, "/opt/trn_rl_repo")

import numpy as np
import ml_dtypes

B, LQ, LK, E, H = 2, 1024, 10000, 256, 8
D = 32            # head dim
HPC = 2           # heads per core
DC = D * HPC      # 64 projected dims per core
LKP = 10240       # LK padded to a multiple of 512
NKT = LKP // 128  # 80 k-tiles
NKT_RUN = 79      # tile 79 is all padding (LK=10000 < 79*128)
NCH = LKP // 512  # 20 dma chunks

# Schraudolph exp constants (bf16 bits via int16).
SCH_A = float(2.0**7 / np.log(2.0))
SCH_B = float(127.0 * 128.0 - 7.35)

_CACHE = {}


def _build():
    import concourse.bacc as bacc
    import concourse.tile as tile
    from concourse import mybir

    F32 = mybir.dt.float32
    F16 = mybir.dt.float16
    BF16 = mybir.dt.bfloat16
    I16 = mybir.dt.int16
    AF = mybir.ActivationFunctionType
    ALU = mybir.AluOpType

    nc = bacc.Bacc("TRN2", target_bir_lowering=False)

    XQT = nc.dram_tensor("xqt", [128, 2, LQ], F16, kind="ExternalInput")
    XKT = nc.dram_tensor("xkt", [128, 2, LKP], F16, kind="ExternalInput")
    # all small constants packed into one byte tensor -> one DMA
    # layout (bytes): [0:512) wkv f16, [512:768) wq f16, [768:896) identb
    # f16, [896:1152) identbb bf16, [1152:1664) ident f32, [1664:1668) bq
    # f32 (rows 0-63), [1668:2692) wo bf16 (rows 0-31)
    # ... [2692:3204) wrm f32 (dense random data for the HAM feed)
    WPK = nc.dram_tensor("wpack", [128, 3204], mybir.dt.uint8,
                         kind="ExternalInput")
    # partial output, transposed: rows = embed dim, cols = query position
    OUT = nc.dram_tensor("out_t", [E, LQ], F32, kind="ExternalOutput")

    with tile.TileContext(nc) as tc:
        with (
            tc.tile_pool(name="singles", bufs=1) as sg,
            tc.tile_pool(name="stt", bufs=8) as stp,
            tc.tile_pool(name="ktp", bufs=2) as ktp,
            tc.tile_pool(name="avps", bufs=1, space="PSUM") as avp,
        ):
            # ---- inputs, ordered by first use (DMAs serialize on Sync) --
            wpk = sg.tile([128, 3204], mybir.dt.uint8, tag="wpk")
            nc.sync.dma_start(out=wpk, in_=WPK[:, :])
            xkT = sg.tile([128, 2, LKP], F16, tag="xkT")
            nc.sync.dma_start(out=xkT[:, :, 0:512], in_=XKT[:, :, 0:512])
            xqT = sg.tile([128, 2, LQ], F16, tag="xqT")
            nc.sync.dma_start(out=xqT, in_=XQT[:, :, :])
            wkv_r = wpk[:, 0:512].bitcast(F16).rearrange(
                "p (e c) -> p e c", e=2, c=128)
            wq_r = wpk[:, 512:768].bitcast(F16).rearrange(
                "p (e c) -> p e c", e=2, c=DC)
            identb = wpk[:, 768:896].bitcast(F16)
            identbb = wpk[:, 896:1152].bitcast(BF16)
            ident = wpk[:, 1152:1664].bitcast(F32)
            bq_sb = wpk[0:64, 1664:1668].bitcast(F32)
            wo_r = wpk[0:32, 1668:2692].bitcast(BF16).rearrange(
                "p (e c) -> p e c", e=2, c=E)

            # warm the ACT exp table before the steady loop (~2.7us load)
            dumm = sg.tile([64, 1], BF16, tag="dumm")
            nc.scalar.activation(dumm, bq_sb, AF.Exp)

            # random-data fp32 view for the HAM warm matmuls: the activity
            # monitor tracks actual fp32-path array toggling; 16-bit
            # matmuls never register, so the gate must be fed explicitly.
            # A wpack view (no copy) so the flip burst starts right after
            # the first DMA lands.
            wrm = wpk[:, 2692:3204].bitcast(F32)

            qT = sg.tile([64, LQ], F16, tag="qT")
            v_aug = sg.tile([128, NKT * 66], BF16, tag="vaug")
            # ones columns of v_aug (softmax-denominator trick)
            nc.vector.memset(
                v_aug[:, :].rearrange("p (k o) -> p k o", o=33)[:, :, 32:33],
                1.0)
            zz = sg.tile([1, 640], BF16, tag="zz")
            nc.vector.memset(zz, 0.0)

            av = {}
            kts = {}
            pending = []
            n_grp = [0]

            def warm(st, n):
                # HAM clock-gate feed: one plain-fp32 matmul inside every
                # ~3.4us window holds K=8/8.  Writes into a PSUM slot the
                # next start=True matmul overwrites, so it costs nothing.
                for _ in range(n):
                    nc.tensor.matmul(st[0:32, 0:128], wrm[:, 0:32],
                                     wrm[:, :], start=True, stop=True,
                                     skip_group_check=True)

            with (
                tc.tile_pool(name="stg0", bufs=5, space="PSUM") as ps0,
                tc.tile_pool(name="kvp", bufs=1, space="PSUM") as kvp,
            ):
                # ~4us dense fp32 burst to flip the HAM gate to K=8/8;
                # overlaps the prologue DMA chain (only needs xqT)
                wb = ps0.tile([128, 512], F32, tag="stg", name="warmb")
                warm(wb, 9)

                def dma_chunk(c):
                    cs = slice(c * 512, (c + 1) * 512)
                    nc.sync.dma_start(out=xkT[:, :, cs], in_=XKT[:, :, cs])

                def kv_stage(c):
                    # k/v projection, one stationary [Wk | Wv], one fp16
                    # PSUM evacuation for both k (rows 0-63) and v^T
                    cs = slice(c * 512, (c + 1) * 512)
                    kv = kvp.tile([128, 512], F32, tag="kv", name=f"kv{c}")
                    for e in range(2):
                        nc.tensor.matmul(kv, wkv_r[:, e, :], xkT[:, e, cs],
                                         start=(e == 0), stop=(e == 1))
                    kvt = ktp.tile([128, 512], F16, tag="kvt",
                                   name=f"kvt{c}")
                    nc.scalar.copy(kvt, kv)
                    kts[c] = kvt

                def v_stage(c):
                    # v^T -> v via PE transposes (PSUM bank shared with kv
                    # via the pool tag), then strided bf16 copy
                    kvt = kts[c]
                    vpsf = kvp.tile([128, 512], F32, tag="kv",
                                    name=f"vps{c}")
                    vps = vpsf[:, 0:128].bitcast(F16)
                    for m in range(4):
                        nc.tensor.transpose(
                            vps[:, m * 64:(m + 1) * 64],
                            kvt[64:128, m * 128:(m + 1) * 128],
                            identb[64:128, :])
                    nc.vector.tensor_copy(
                        v_aug[:, c * 264:(c + 1) * 264].rearrange(
                            "p (k t o) -> p k t o", t=2, o=33)[:, :, :, 0:32],
                        vps[:, :].rearrange("p (k t d) -> p k t d", t=2, d=32))

                def flush_av(depth):
                    if len(pending) < depth:
                        return
                    kt, qc, sTs = pending.pop(0)
                    for h in range(HPC):
                        # the K=1 zero matmul below initialized the whole
                        # bank, so every accumulation is start=False
                        nc.tensor.matmul(
                            av[qc][64 * h:64 * h + 33, :],
                            v_aug[:, kt * 66 + 33 * h:kt * 66 + 33 * h + 33],
                            sTs[h],
                            start=False, stop=(kt == NKT_RUN - 1),
                            skip_group_check=True)

                def emit_group(kt, qc, ktile):
                    g = n_grp[0]
                    n_grp[0] += 1
                    sts = [ps0.tile([128, 512], F32, tag="stg",
                                    name=f"stg{g}h{h}") for h in range(HPC)]
                    if g % 4 == 0:
                        warm(sts[0], 1)
                    for h in range(HPC):
                        nc.tensor.matmul(
                            sts[h],
                            ktile[32 * h:32 * h + 32,
                                  (kt % 4) * 128:(kt % 4 + 1) * 128],
                            qT[32 * h:32 * h + 32, qc * 512:(qc + 1) * 512],
                            start=True, stop=True)
                    sTs = []
                    for h in range(HPC):
                        sT = stp.tile([128, 512], BF16, tag="sT",
                                      name=f"sT{g}h{h}")
                        if (kt + qc + h) % 2 == 0:
                            nc.scalar.activation(sT, sts[h], AF.Exp)
                        else:
                            nc.vector.tensor_scalar(
                                out=sT.bitcast(I16), in0=sts[h],
                                scalar1=SCH_A, scalar2=SCH_B,
                                op0=ALU.mult, op1=ALU.add)
                        sTs.append(sT)
                    flush_av(3)
                    pending.append((kt, qc, sTs))

                # ---- prologue ----
                for c in range(1, NCH):
                    dma_chunk(c)

                # q projection (borrows stg psum tiles)
                for qc in range(2):
                    qp = ps0.tile([128, 512], F32, tag="stg",
                                  name=f"stq{qc}")
                    for e in range(2):
                        nc.tensor.matmul(qp[0:64, :], wq_r[:, e, :],
                                         xqT[:, e, qc * 512:(qc + 1) * 512],
                                         start=(e == 0), stop=(e == 1))
                    nc.vector.tensor_scalar_add(
                        qT[:, qc * 512:(qc + 1) * 512], qp[0:64, :],
                        bq_sb[:, 0:1])

                kv_stage(0)
                v_stage(0)
                av[0] = avp.tile([128, 512], F32, tag="av_0", name="av_q0")
                av[1] = avp.tile([128, 512], F32, tag="av_1", name="av_q1")
                for qc in range(2):
                    # zero-fill the whole accumulator bank (K=1 matmul of
                    # zeros) so has_written covers all 128 partitions
                    nc.tensor.matmul(
                        av[qc][0:128, :], zz[0:1, 0:128], zz[0:1, 128:640],
                        start=True, stop=False, skip_group_check=True)

                # ---- steady state: software-pipelined by one chunk ----
                for c in range(NCH):
                    ktile = kts.pop(c)
                    i = 0
                    for j in range(4):
                        if c * 4 + j >= NKT_RUN:
                            break
                        for qc in range(2):
                            emit_group(c * 4 + j, qc, ktile)
                            if i == 3 and c + 1 < NCH:
                                kv_stage(c + 1)
                            if i == 5 and c + 1 < NCH:
                                v_stage(c + 1)
                            i += 1
                while pending:
                    flush_av(1)

            # =========== normalize + output projection ----
            attnT = sg.tile([32, 2, LQ], BF16, tag="attnT")
            out_sb = [sg.tile([128, LQ], F32, tag=f"out{e}", name=f"out{e}")
                      for e in range(2)]
            rT = sg.tile([128, 16], BF16, tag="rT")
            avs_t = {}

            with tc.tile_pool(name="scp", bufs=2, space="PSUM") as scp:
                # evacuate the [33, 512] accumulator slabs to partition
                # base 0 (rows 0-31 = dims, row 32 = denominator)
                for qc in range(2):
                    avs = sg.tile([33, 1024], BF16, tag=f"avs{qc}",
                                  name=f"avs{qc}")
                    for h in range(HPC):
                        if (qc + h) % 2 == 0:
                            nc.scalar.copy(avs[:, h * 512:(h + 1) * 512],
                                           av[qc][64 * h:64 * h + 33, :])
                        else:
                            nc.vector.tensor_copy(
                                avs[:, h * 512:(h + 1) * 512],
                                av[qc][64 * h:64 * h + 33, :])
                    avs_t[qc] = avs
                    warm(av[qc], 2)

                # transpose just the denominator rows into [128, 16] so
                # the reciprocal runs wide on the DVE
                # bf16 PSUM writes need 4-byte alignment: use even columns
                avT = scp.tile([128, 32], BF16, tag="avT", name="avT")
                for qc in range(2):
                    if qc == 1:
                        warm(av[0], 1)
                    for h in range(HPC):
                        for j in range(4):
                            idx = (qc * 2 + h) * 4 + j
                            nc.tensor.transpose(
                                avT[:, 2 * idx:2 * idx + 1],
                                avs_t[qc][32:33, h * 512 + j * 128:
                                          h * 512 + (j + 1) * 128],
                                identbb[32:33, 32:33])
                with nc.allow_low_precision(
                        reason="bf16 denominators: 0.4% rel, within budget"):
                    nc.vector.reciprocal(
                        rT, avT.rearrange("p (m o) -> p m o", o=2)[:, :, 0])

                for qc in range(2):
                    for h in range(HPC):
                        rb = scp.tile([32, 512], F32, tag="rb",
                                      name=f"rb{qc}{h}")
                        warm(rb, 1)
                        for j in range(4):
                            idx = (qc * 2 + h) * 4 + j
                            nc.tensor.matmul(
                                rb[:, j * 128:(j + 1) * 128],
                                rT[:, idx:idx + 1].broadcast_to((128, 32)),
                                identbb, start=True, stop=True)
                        nc.vector.tensor_mul(
                            attnT[:, h, qc * 512:(qc + 1) * 512],
                            avs_t[qc][0:32, h * 512:(h + 1) * 512], rb)

                for ec in range(2):
                    for qc in range(2):
                        po = scp.tile([128, 512], F32, tag="po",
                                      name=f"po{ec}{qc}")
                        if qc == 0:
                            warm(po, 1)
                        for h in range(HPC):
                            nc.tensor.matmul(
                                po, wo_r[:, h, ec * 128:(ec + 1) * 128],
                                attnT[:, h, qc * 512:(qc + 1) * 512],
                                start=(h == 0), stop=(h == 1))
                        if qc == 0:
                            nc.scalar.copy(out_sb[ec][:, 0:512], po)
                        else:
                            nc.vector.tensor_copy(out_sb[ec][:, 512:1024], po)
                    nc.sync.dma_start(out=OUT[ec * 128:(ec + 1) * 128, :],
                                      in_=out_sb[ec])

    nc.compile()
    return nc


def _get_nc():
    if "nc" not in _CACHE:
        _CACHE["nc"] = _build()
    return _CACHE["nc"]


def _host_in_maps(bev_emb, queries, Wq, bq, Wk, bk, Wv, bv, Wo, bo):
    bev_emb = np.asarray(bev_emb, dtype=np.float32)
    queries = np.asarray(queries, dtype=np.float32)
    Wq = np.asarray(Wq, dtype=np.float32)
    bq = np.asarray(bq, dtype=np.float32)
    Wk = np.asarray(Wk, dtype=np.float32)
    bk = np.asarray(bk, dtype=np.float32)
    Wv = np.asarray(Wv, dtype=np.float32)
    bv = np.asarray(bv, dtype=np.float32)
    Wo = np.asarray(Wo, dtype=np.float32)
    bo = np.asarray(bo, dtype=np.float32)

    BF = ml_dtypes.bfloat16
    ident = np.eye(128, dtype=np.float32)
    identb = np.zeros((128, 64), dtype=np.float16)
    identb[64:128] = np.eye(64, dtype=np.float16)
    identbb = np.eye(128, dtype=BF)

    # host-side layout staging (no flops): transposes + padding + casts
    xqt = []
    xkt = []
    for b in range(B):
        t = np.ascontiguousarray(
            queries[b].T.reshape(2, 128, LQ).transpose(1, 0, 2))
        xqt.append(t.astype(np.float16))
        kp = np.zeros((128, 2, LKP), dtype=np.float16)
        kp[:, :, :LK] = bev_emb[b].T.reshape(2, 128, LK).transpose(
            1, 0, 2).astype(np.float16)
        xkt.append(kp)

    in_maps = []
    for c in range(8):
        b, hp = c // 4, c % 4
        hs = slice(hp * DC, (hp + 1) * DC)
        wkv = np.concatenate([Wk[:, hs], Wv[:, hs]], axis=1)  # [256, 128]
        wpk = np.zeros((128, 3204), np.uint8)
        wpk[:, 0:512] = np.ascontiguousarray(
            wkv.reshape(2, 128, 128).transpose(1, 0, 2)).astype(
                np.float16).view(np.uint8).reshape(128, 512)
        wpk[:, 512:768] = np.ascontiguousarray(
            Wq[:, hs].reshape(2, 128, DC).transpose(1, 0, 2)).astype(
                np.float16).view(np.uint8).reshape(128, 256)
        wpk[:, 768:896] = identb.view(np.uint8).reshape(128, 128)
        wpk[:, 896:1152] = identbb.view(np.uint8).reshape(128, 256)
        wpk[:, 1152:1664] = ident.view(np.uint8).reshape(128, 512)
        wpk[0:64, 1664:1668] = np.ascontiguousarray(
            bq[hs]).astype(np.float32).view(np.uint8).reshape(64, 4)
        wpk[0:32, 1668:2692] = np.ascontiguousarray(
            Wo[hs, :].reshape(2, 32, E).transpose(1, 0, 2)).astype(
                BF).view(np.uint8).reshape(32, 1024)
        wpk[:, 2692:3204] = np.random.default_rng(7).standard_normal(
            (128, 128)).astype(np.float32).view(np.uint8).reshape(128, 512)
        in_maps.append({
            "xqt": xqt[b],
            "xkt": xkt[b],
            "wpack": wpk,
        })

    return in_maps


def kernel(bev_emb, queries, Wq, bq, Wk, bk, Wv, bv, Wo, bo):
    from concourse.bass_utils import run_bass_kernel_spmd

    in_maps = _host_in_maps(bev_emb, queries, Wq, bq, Wk, bk, Wv, bv, Wo, bo)
    nc = _get_nc()
    _CACHE["last_in_maps"] = in_maps
    res = run_bass_kernel_spmd(nc, in_maps, list(range(8)))
    _CACHE["last_result"] = res

    out = np.zeros((B, LQ, E), dtype=np.float32)
    for c in range(8):
        out[c // 4] += res.results[c]["out_t"].T
    # bk drops out of softmax exactly; bv rides through attention into the
    # output projection: out += bv @ Wo.  Both folded into the host bias.
    out += bo + bv @ Wo
    return out


# revision 25
# speedup vs baseline: 1.1729x; 1.0337x over previous
"""Cross-attention kernel for Trainium2, SPMD over 8 NeuronCores.

Problem: B=2, LQ=1024, LK=10000, E=256, H=8 heads of D=32.
  q = queries @ Wq + bq ; k = bev @ Wk + bk ; v = bev @ Wv + bv
  out = softmax(q k^T) v  @ Wo + bo

Sharding: core c -> (batch b = c // 4, head-pair hp = c % 4).  Each core
computes attention for its 2 heads of its batch plus the partial output
projection through its 64 rows of Wo.  Host sums the 4 partials per batch
and adds bo (plus the bv @ Wo term, see below).

Structural choices:
  - All hot-loop matmuls are 16-bit: q/k/v in fp16 (energies need the
    mantissa; fp16 streams 1 col/cycle vs 2 for fp32r), softmax weights in
    bf16 (exp values up to e^30 overflow fp16's range).  16-bit matmuls
    also feed the PE activity monitor, so no fp32 HAM-warm matmuls needed.
  - bk is dropped entirely (softmax is invariant to a per-query constant);
    bv rides through attention (weights sum to 1) and is folded into the
    host-side bias as bv @ Wo.  Both exact.
  - The softmax denominator comes from an all-ones column appended to v,
    so it falls out of the same PE matmuls that compute attn @ v.
  - exp() alternates per (kt, qc, head) unit between the Scalar engine
    (exact table exp) and the Vector engine computing a Schraudolph exp:
    bf16 bits as round(x * 2^7/ln2 + (127*2^7 - 7.35)) int16.  The -7.35
    debias makes the approximation mean-preserving so exact and
    approximate tiles mix without tilting the softmax average.
  - Energy PSUM tiles are per-head single banks in a 5-deep ring, so an
    exp only gates its own bank and the engines never co-idle waiting for
    a 2-bank group to drain.  k/v projection PSUM and the v-transpose
    PSUM share one further bank (temporally disjoint, same pool tag).
  - Energy matmuls (K=32) run as concurrent row-packed pairs (heads at PE
    rows 0-31 / 32-63); attn@v pairs are column-packed (output partitions
    0-32 / 64-96).
  - k/v projections share one stationary [Wk | Wv]; a single fp16
    [128, 512] PSUM evacuation per chunk yields both the k tile and the
    v^T tile, keeping the Vector engine free for exp.
  - Reciprocal of the 2048 denominators is done in a [128, 16] layout
    (tiny [1, 128] PE transposes in, 0-stride-broadcast matmuls out)
    instead of [1, 512] rows, which would run 30x slower on the per-lane
    DVE.
"""
import sys

sys.path.insert(0, "/opt/trn_rl_repo")

import numpy as np
import ml_dtypes

B, LQ, LK, E, H = 2, 1024, 10000, 256, 8
D = 32            # head dim
HPC = 2           # heads per core
DC = D * HPC      # 64 projected dims per core
LKP = 10240       # LK padded to a multiple of 512
NKT = LKP // 128  # 80 k-tiles
NKT_RUN = 79      # tile 79 is all padding (LK=10000 < 79*128)
NCH = LKP // 512  # 20 dma chunks

# Schraudolph exp constants (bf16 bits via int16).
SCH_A = float(2.0**7 / np.log(2.0))
SCH_B = float(127.0 * 128.0 - 7.35)

_CACHE = {}


def _build():
    import concourse.bacc as bacc
    import concourse.tile as tile
    from concourse import mybir

    F32 = mybir.dt.float32
    F16 = mybir.dt.float16
    BF16 = mybir.dt.bfloat16
    I16 = mybir.dt.int16
    AF = mybir.ActivationFunctionType
    ALU = mybir.AluOpType

    nc = bacc.Bacc("TRN2", target_bir_lowering=False)

    XQT = nc.dram_tensor("xqt", [128, 2, LQ], F16, kind="ExternalInput")
    XKT = nc.dram_tensor("xkt", [128, 2, LKP], F16, kind="ExternalInput")
    # all small constants packed into one byte tensor -> one DMA
    # layout (bytes): [0:512) wkv f16, [512:768) wq f16, [768:896) identb
    # f16, [896:1152) identbb bf16, [1152:1664) ident f32, [1664:1668) bq
    # f32 (rows 0-63), [1668:2692) wo bf16 (rows 0-31)
    # ... [2692:3204) wrm f32 (dense random data for the HAM feed)
    WPK = nc.dram_tensor("wpack", [128, 3204], mybir.dt.uint8,
                         kind="ExternalInput")
    # partial output, transposed: rows = embed dim, cols = query position
    OUT = nc.dram_tensor("out_t", [E, LQ], F32, kind="ExternalOutput")

    with tile.TileContext(nc) as tc:
        with (
            tc.tile_pool(name="singles", bufs=1) as sg,
            tc.tile_pool(name="stt", bufs=8) as stp,
            tc.tile_pool(name="ktp", bufs=2) as ktp,
            tc.tile_pool(name="avps", bufs=1, space="PSUM") as avp,
        ):
            # ---- inputs, ordered by first use (DMAs serialize on Sync) --
            wpk = sg.tile([128, 3204], mybir.dt.uint8, tag="wpk")
            nc.sync.dma_start(out=wpk, in_=WPK[:, :])
            xkT = sg.tile([128, 2, LKP], F16, tag="xkT")
            nc.sync.dma_start(out=xkT[:, :, 0:512], in_=XKT[:, :, 0:512])
            xqT = sg.tile([128, 2, LQ], F16, tag="xqT")
            nc.sync.dma_start(out=xqT, in_=XQT[:, :, :])
            wkv_r = wpk[:, 0:512].bitcast(F16).rearrange(
                "p (e c) -> p e c", e=2, c=128)
            wq_r = wpk[:, 512:768].bitcast(F16).rearrange(
                "p (e c) -> p e c", e=2, c=DC)
            identb = wpk[:, 768:896].bitcast(F16)
            identbb = wpk[:, 896:1152].bitcast(BF16)
            ident = wpk[:, 1152:1664].bitcast(F32)
            bq_sb = wpk[0:64, 1664:1668].bitcast(F32)
            wo_r = wpk[0:32, 1668:2692].bitcast(BF16).rearrange(
                "p (e c) -> p e c", e=2, c=E)

            # warm the ACT exp table before the steady loop (~2.7us load)
            dumm = sg.tile([64, 1], BF16, tag="dumm")
            nc.scalar.activation(dumm, bq_sb, AF.Exp)

            # random-data fp32 view for the HAM warm matmuls: the activity
            # monitor tracks actual fp32-path array toggling; 16-bit
            # matmuls never register, so the gate must be fed explicitly.
            # A wpack view (no copy) so the flip burst starts right after
            # the first DMA lands.
            wrm = wpk[:, 2692:3204].bitcast(F32)

            qT = sg.tile([64, LQ], F16, tag="qT")
            v_aug = sg.tile([128, NKT * 66], BF16, tag="vaug")
            # ones columns of v_aug (softmax-denominator trick)
            nc.vector.memset(
                v_aug[:, :].rearrange("p (k o) -> p k o", o=33)[:, :, 32:33],
                1.0)
            zz = sg.tile([1, 640], BF16, tag="zz")
            nc.vector.memset(zz, 0.0)

            av = {}
            kts = {}
            pending = []
            n_grp = [0]

            def warm(st, n, cols=128):
                # HAM clock-gate feed: one plain-fp32 matmul inside every
                # ~3.4us window holds K=8/8.  Writes into a PSUM slot the
                # next start=True matmul overwrites, so it costs nothing.
                for _ in range(n):
                    nc.tensor.matmul(st[0:32, 0:cols], wrm[:, 0:32],
                                     wrm[:, 0:cols], start=True, stop=True,
                                     skip_group_check=True)

            with (
                tc.tile_pool(name="stg0", bufs=5, space="PSUM") as ps0,
                tc.tile_pool(name="kvp", bufs=1, space="PSUM") as kvp,
            ):
                # ~4us dense fp32 burst to flip the HAM gate to K=8/8;
                # overlaps the prologue DMA chain (only needs xqT)
                wb = ps0.tile([128, 512], F32, tag="stg", name="warmb")
                warm(wb, 9)

                def dma_chunk(c):
                    cs = slice(c * 512, (c + 1) * 512)
                    nc.sync.dma_start(out=xkT[:, :, cs], in_=XKT[:, :, cs])

                def kv_stage(c):
                    # k/v projection, one stationary [Wk | Wv], one fp16
                    # PSUM evacuation for both k (rows 0-63) and v^T
                    cs = slice(c * 512, (c + 1) * 512)
                    kv = kvp.tile([128, 512], F32, tag="kv", name=f"kv{c}")
                    for e in range(2):
                        nc.tensor.matmul(kv, wkv_r[:, e, :], xkT[:, e, cs],
                                         start=(e == 0), stop=(e == 1))
                    kvt = ktp.tile([128, 512], F16, tag="kvt",
                                   name=f"kvt{c}")
                    nc.scalar.copy(kvt, kv)
                    kts[c] = kvt

                def v_stage(c):
                    # v^T -> v via PE transposes (PSUM bank shared with kv
                    # via the pool tag), then strided bf16 copy
                    kvt = kts[c]
                    vpsf = kvp.tile([128, 512], F32, tag="kv",
                                    name=f"vps{c}")
                    vps = vpsf[:, 0:128].bitcast(F16)
                    for m in range(4):
                        nc.tensor.transpose(
                            vps[:, m * 64:(m + 1) * 64],
                            kvt[64:128, m * 128:(m + 1) * 128],
                            identb[64:128, :])
                    nc.vector.tensor_copy(
                        v_aug[:, c * 264:(c + 1) * 264].rearrange(
                            "p (k t o) -> p k t o", t=2, o=33)[:, :, :, 0:32],
                        vps[:, :].rearrange("p (k t d) -> p k t d", t=2, d=32))

                def flush_av(depth):
                    if len(pending) < depth:
                        return
                    kt, qc, sTs = pending.pop(0)
                    for h in range(HPC):
                        # the K=1 zero matmul below initialized the whole
                        # bank, so every accumulation is start=False
                        nc.tensor.matmul(
                            av[qc][64 * h:64 * h + 33, :],
                            v_aug[:, kt * 66 + 33 * h:kt * 66 + 33 * h + 33],
                            sTs[h],
                            start=False, stop=(kt == NKT_RUN - 1),
                            skip_group_check=True)

                def emit_group(kt, qc, ktile):
                    g = n_grp[0]
                    n_grp[0] += 1
                    sts = [ps0.tile([128, 512], F32, tag="stg",
                                    name=f"stg{g}h{h}") for h in range(HPC)]
                    if g % 4 == 0:
                        warm(sts[0], 1, cols=64)
                    for h in range(HPC):
                        nc.tensor.matmul(
                            sts[h],
                            ktile[32 * h:32 * h + 32,
                                  (kt % 4) * 128:(kt % 4 + 1) * 128],
                            qT[32 * h:32 * h + 32, qc * 512:(qc + 1) * 512],
                            start=True, stop=True)
                    sTs = []
                    for h in range(HPC):
                        sT = stp.tile([128, 512], BF16, tag="sT",
                                      name=f"sT{g}h{h}")
                        if (kt + qc + h) % 2 == 0:
                            nc.scalar.activation(sT, sts[h], AF.Exp)
                        else:
                            nc.vector.tensor_scalar(
                                out=sT.bitcast(I16), in0=sts[h],
                                scalar1=SCH_A, scalar2=SCH_B,
                                op0=ALU.mult, op1=ALU.add)
                        sTs.append(sT)
                    flush_av(3)
                    pending.append((kt, qc, sTs))

                # ---- prologue ----
                for c in range(1, NCH):
                    dma_chunk(c)

                # q projection (borrows stg psum tiles)
                for qc in range(2):
                    qp = ps0.tile([128, 512], F32, tag="stg",
                                  name=f"stq{qc}")
                    for e in range(2):
                        nc.tensor.matmul(qp[0:64, :], wq_r[:, e, :],
                                         xqT[:, e, qc * 512:(qc + 1) * 512],
                                         start=(e == 0), stop=(e == 1))
                    nc.vector.tensor_scalar_add(
                        qT[:, qc * 512:(qc + 1) * 512], qp[0:64, :],
                        bq_sb[:, 0:1])

                kv_stage(0)
                v_stage(0)
                av[0] = avp.tile([128, 512], F32, tag="av_0", name="av_q0")
                av[1] = avp.tile([128, 512], F32, tag="av_1", name="av_q1")
                for qc in range(2):
                    # zero-fill the whole accumulator bank (K=1 matmul of
                    # zeros) so has_written covers all 128 partitions
                    nc.tensor.matmul(
                        av[qc][0:128, :], zz[0:1, 0:128], zz[0:1, 128:640],
                        start=True, stop=False, skip_group_check=True)

                # ---- steady state: software-pipelined by one chunk ----
                for c in range(NCH):
                    ktile = kts.pop(c)
                    i = 0
                    for j in range(4):
                        if c * 4 + j >= NKT_RUN:
                            break
                        for qc in range(2):
                            emit_group(c * 4 + j, qc, ktile)
                            if i == 3 and c + 1 < NCH:
                                kv_stage(c + 1)
                            if i == 5 and c + 1 < NCH:
                                v_stage(c + 1)
                            i += 1
                while pending:
                    flush_av(1)

            # =========== normalize + output projection ----
            attnT = sg.tile([32, 2, LQ], BF16, tag="attnT")
            out_sb = [sg.tile([128, LQ], F32, tag=f"out{e}", name=f"out{e}")
                      for e in range(2)]
            rT = sg.tile([128, 16], BF16, tag="rT")
            avs_t = {}

            with tc.tile_pool(name="scp", bufs=2, space="PSUM") as scp:
                # evacuate the [33, 512] accumulator slabs to partition
                # base 0 (rows 0-31 = dims, row 32 = denominator)
                for qc in range(2):
                    avs = sg.tile([33, 1024], BF16, tag=f"avs{qc}",
                                  name=f"avs{qc}")
                    for h in range(HPC):
                        if (qc + h) % 2 == 0:
                            nc.scalar.copy(avs[:, h * 512:(h + 1) * 512],
                                           av[qc][64 * h:64 * h + 33, :])
                        else:
                            nc.vector.tensor_copy(
                                avs[:, h * 512:(h + 1) * 512],
                                av[qc][64 * h:64 * h + 33, :])
                    avs_t[qc] = avs
                    warm(av[qc], 2)

                # transpose just the denominator rows into [128, 16] so
                # the reciprocal runs wide on the DVE
                # bf16 PSUM writes need 4-byte alignment: use even columns
                avT = scp.tile([128, 32], BF16, tag="avT", name="avT")
                for qc in range(2):
                    if qc == 1:
                        warm(av[0], 1)
                    for h in range(HPC):
                        for j in range(4):
                            idx = (qc * 2 + h) * 4 + j
                            nc.tensor.transpose(
                                avT[:, 2 * idx:2 * idx + 1],
                                avs_t[qc][32:33, h * 512 + j * 128:
                                          h * 512 + (j + 1) * 128],
                                identbb[32:33, 32:33])
                with nc.allow_low_precision(
                        reason="bf16 denominators: 0.4% rel, within budget"):
                    nc.vector.reciprocal(
                        rT, avT.rearrange("p (m o) -> p m o", o=2)[:, :, 0])

                for qc in range(2):
                    for h in range(HPC):
                        rb = scp.tile([32, 512], F32, tag="rb",
                                      name=f"rb{qc}{h}")
                        warm(rb, 1)
                        for j in range(4):
                            idx = (qc * 2 + h) * 4 + j
                            nc.tensor.matmul(
                                rb[:, j * 128:(j + 1) * 128],
                                rT[:, idx:idx + 1].broadcast_to((128, 32)),
                                identbb, start=True, stop=True)
                        nc.vector.tensor_mul(
                            attnT[:, h, qc * 512:(qc + 1) * 512],
                            avs_t[qc][0:32, h * 512:(h + 1) * 512], rb)

                for ec in range(2):
                    for qc in range(2):
                        po = scp.tile([128, 512], F32, tag="po",
                                      name=f"po{ec}{qc}")
                        if qc == 0:
                            warm(po, 1)
                        for h in range(HPC):
                            nc.tensor.matmul(
                                po, wo_r[:, h, ec * 128:(ec + 1) * 128],
                                attnT[:, h, qc * 512:(qc + 1) * 512],
                                start=(h == 0), stop=(h == 1))
                        if qc == 0:
                            nc.scalar.copy(out_sb[ec][:, 0:512], po)
                        else:
                            nc.vector.tensor_copy(out_sb[ec][:, 512:1024], po)
                    nc.sync.dma_start(out=OUT[ec * 128:(ec + 1) * 128, :],
                                      in_=out_sb[ec])

    nc.compile()
    return nc


def _get_nc():
    if "nc" not in _CACHE:
        _CACHE["nc"] = _build()
    return _CACHE["nc"]


def _host_in_maps(bev_emb, queries, Wq, bq, Wk, bk, Wv, bv, Wo, bo):
    bev_emb = np.asarray(bev_emb, dtype=np.float32)
    queries = np.asarray(queries, dtype=np.float32)
    Wq = np.asarray(Wq, dtype=np.float32)
    bq = np.asarray(bq, dtype=np.float32)
    Wk = np.asarray(Wk, dtype=np.float32)
    bk = np.asarray(bk, dtype=np.float32)
    Wv = np.asarray(Wv, dtype=np.float32)
    bv = np.asarray(bv, dtype=np.float32)
    Wo = np.asarray(Wo, dtype=np.float32)
    bo = np.asarray(bo, dtype=np.float32)

    BF = ml_dtypes.bfloat16
    ident = np.eye(128, dtype=np.float32)
    identb = np.zeros((128, 64), dtype=np.float16)
    identb[64:128] = np.eye(64, dtype=np.float16)
    identbb = np.eye(128, dtype=BF)

    # host-side layout staging (no flops): transposes + padding + casts
    xqt = []
    xkt = []
    for b in range(B):
        t = np.ascontiguousarray(
            queries[b].T.reshape(2, 128, LQ).transpose(1, 0, 2))
        xqt.append(t.astype(np.float16))
        kp = np.zeros((128, 2, LKP), dtype=np.float16)
        kp[:, :, :LK] = bev_emb[b].T.reshape(2, 128, LK).transpose(
            1, 0, 2).astype(np.float16)
        xkt.append(kp)

    in_maps = []
    for c in range(8):
        b, hp = c // 4, c % 4
        hs = slice(hp * DC, (hp + 1) * DC)
        wkv = np.concatenate([Wk[:, hs], Wv[:, hs]], axis=1)  # [256, 128]
        wpk = np.zeros((128, 3204), np.uint8)
        wpk[:, 0:512] = np.ascontiguousarray(
            wkv.reshape(2, 128, 128).transpose(1, 0, 2)).astype(
                np.float16).view(np.uint8).reshape(128, 512)
        wpk[:, 512:768] = np.ascontiguousarray(
            Wq[:, hs].reshape(2, 128, DC).transpose(1, 0, 2)).astype(
                np.float16).view(np.uint8).reshape(128, 256)
        wpk[:, 768:896] = identb.view(np.uint8).reshape(128, 128)
        wpk[:, 896:1152] = identbb.view(np.uint8).reshape(128, 256)
        wpk[:, 1152:1664] = ident.view(np.uint8).reshape(128, 512)
        wpk[0:64, 1664:1668] = np.ascontiguousarray(
            bq[hs]).astype(np.float32).view(np.uint8).reshape(64, 4)
        wpk[0:32, 1668:2692] = np.ascontiguousarray(
            Wo[hs, :].reshape(2, 32, E).transpose(1, 0, 2)).astype(
                BF).view(np.uint8).reshape(32, 1024)
        wpk[:, 2692:3204] = np.random.default_rng(7).standard_normal(
            (128, 128)).astype(np.float32).view(np.uint8).reshape(128, 512)
        in_maps.append({
            "xqt": xqt[b],
            "xkt": xkt[b],
            "wpack": wpk,
        })

    return in_maps


def kernel(bev_emb, queries, Wq, bq, Wk, bk, Wv, bv, Wo, bo):
    from concourse.bass_utils import run_bass_kernel_spmd

    in_maps = _host_in_maps(bev_emb, queries, Wq, bq, Wk, bk, Wv, bv, Wo, bo)
    nc = _get_nc()
    _CACHE["last_in_maps"] = in_maps
    res = run_bass_kernel_spmd(nc, in_maps, list(range(8)))
    _CACHE["last_result"] = res

    out = np.zeros((B, LQ, E), dtype=np.float32)
    for c in range(8):
        out[c // 4] += res.results[c]["out_t"].T
    # bk drops out of softmax exactly; bv rides through attention into the
    # output projection: out += bv @ Wo.  Both folded into the host bias.
    out += bo + bv @ Wo
    return out
